# revision 12
# baseline (speedup 1.0000x reference)
"""Trainium2 Bass kernel for the dilated residual point-cloud block.

Contract: kernel(**inputs) takes FULL inputs (feature [2,32,16384,1] f32,
xyz [2,16384,3] f32, neigh_idx [2,16384,16] int64, params pytree) and
returns the FULL output [2,128,16384,1] f32. Internally shards the 16384
points across 8 NeuronCores (2048 points per core per batch element).
"""

import os
import numpy as np

import concourse.bacc as bacc
import concourse.bass as bass
import concourse.tile as tile
from concourse import mybir
from concourse.bass_utils import run_bass_kernel_spmd

F16 = mybir.dt.float16
F32 = mybir.dt.float32
I16 = mybir.dt.int16

EPS = 1e-5
B, N, K = 2, 16384, 16
D_IN, C1, D_OUT, C2 = 32, 32, 64, 128
NCORES = 8
NS = N // NCORES            # points per core per batch = 2048
TP = 128                    # points per tile
NT = NS // TP               # tiles per batch = 16
STT = 4                     # tiles per supertile
NST = NT // STT             # supertiles per batch = 4
TABC = 128                  # fp16 channels per table row (256B)
NKC = TP * K                # nk columns per tile = 2048
QC = NKC // 4               # quarter columns = 512


def _fold(bn):
    s = np.asarray(bn["g"]) / np.sqrt(np.asarray(bn["v"]) + EPS)
    t = np.asarray(bn["b"]) - np.asarray(bn["m"]) * s
    return s.astype(np.float64), t.astype(np.float64)


def _bd(blocks):
    """Block-diagonal stack of equal [ci, co] blocks -> [sum ci, sum co]."""
    ci = sum(b.shape[0] for b in blocks)
    co = sum(b.shape[1] for b in blocks)
    out = np.zeros((ci, co), np.float64)
    ri = co_i = 0
    for b in blocks:
        out[ri:ri + b.shape[0], co_i:co_i + b.shape[1]] = b
        ri += b.shape[0]
        co_i += b.shape[1]
    return out


def _prep_consts(params):
    """Fold BNs, build device weight/bias arrays."""
    p = params
    c = {}
    # mlp1: relu(bn(W1 @ f))
    s1, t1 = _fold(p["mlp1"]["bn"])
    W1 = np.asarray(p["mlp1"]["W"]) * s1[:, None]          # [32, 32]
    c["w1bd"] = _bd([W1.T] * 4).astype(np.float16)          # [128,128]
    c["w1T"] = W1.T.astype(np.float16)                      # [32, 32] (self pc)
    c["b1"] = np.tile(t1, 4).astype(np.float32)             # [128]
    c["b1s"] = t1.astype(np.float32)                        # [32]
    # fc (no bias/bn)
    fcW = np.asarray(p["fc_W"])                             # [32, 32]
    c["fcbd"] = _bd([fcW.T] * 4).astype(np.float16)         # [128,128]
    # weightnet
    wn = p["wn"]
    sw, tw = zip(*[_fold(lp["bn"]) for lp in wn])
    Wn = [np.asarray(lp["W"]) * s[:, None] for lp, s in zip(wn, sw)]
    bn_ = [s * (np.asarray(lp["b"]) - 0) + 0 for lp, s in zip(wn, sw)]  # placeholder
    bw = [s * np.asarray(lp["b"]) + t for lp, s, t in zip(wn, sw, tw)]
    # wn1: [8,3] -> bd over 4 col-quarters with 4-row input blocks (3 real + 1 pad)
    w1 = Wn[0]                                              # [8, 3]
    wn1bd = np.zeros((128, 128), np.float64)
    bwn1 = np.zeros(128, np.float64)
    for q in range(4):
        wn1bd[32 * q:32 * q + 3, 32 * q:32 * q + 8] = w1.T
        bwn1[32 * q:32 * q + 8] = bw[0]
    c["wn1bd"] = wn1bd.astype(np.float16)
    c["bwn1"] = bwn1.astype(np.float32)                     # [128]
    c["wn1T"] = np.vstack([w1.T, np.zeros((1, 8))]).astype(np.float16)  # [4, 8] self
    c["bwn1s"] = bw[0].astype(np.float32)                   # [8] (unused; kept)
    wn2bd = np.zeros((128, 128), np.float64)
    for q in range(4):
        wn2bd[32 * q:32 * q + 8, 32 * q:32 * q + 8] = Wn[1].T
    c["wn2bd"] = wn2bd.astype(np.float16)                   # [32, 128]
    bwn2 = np.zeros(128, np.float64)
    for q in range(4):
        bwn2[32 * q:32 * q + 8] = bw[1]
    c["bwn2"] = bwn2.astype(np.float32)                     # [128]
    c["w3rep"] = Wn[2].T.astype(np.float16)                 # [8, 16]
    c["bwn3row"] = bw[2].astype(np.float16)[None, :]        # [1, 16]
    # mlp chain
    sm_, tm_ = zip(*[_fold(lp["bn"]) for lp in p["mlp"]])
    Wm = [np.asarray(lp["W"]) * s[:, None] for lp, s in zip(p["mlp"], sm_)]
    bm = [s * np.asarray(lp["b"]) + t for lp, s, t in zip(p["mlp"], sm_, tm_)]
    l1 = _bd([Wm[0].T] * 2)                                 # [64, 128]
    c["l1bd"] = np.vstack([l1, l1]).astype(np.float16)      # [128,128] replicated
    c["bL1"] = np.tile(bm[0], 2).astype(np.float32)         # [128]
    c["l2bd"] = _bd([Wm[1].T] * 2).astype(np.float16)       # [128,128]
    c["bL2"] = np.tile(bm[1], 2).astype(np.float32)
    c["l3bd"] = _bd([Wm[2].T] * 2).astype(np.float16)
    c["bL3"] = np.tile(bm[2], 2).astype(np.float32)         # [128]
    c["w4aug"] = np.vstack([Wm[3].T, bm[3][None, :]]).astype(np.float16)  # [65, 128]
    # lin (+bn_lin folded); lin bias folded through mlp2
    sl, tl = _fold(p["bn_lin"])
    linW = np.asarray(p["lin_W"]) * sl[:, None]             # [128, 2048]
    lin_b = sl * np.asarray(p["lin_b"]) + tl                # [128]
    A = linW.reshape(C2, C2, 16)                            # [o, c, w]
    awt = np.zeros((C2, 16 * C2), np.float64)               # [c, 128w + o]
    for w in range(16):
        awt[:, 128 * w:128 * (w + 1)] = A[:, :, w].T
    c["awt"] = awt.astype(np.float16)
    # mlp2 (no relu) + lin bias fold
    s2, t2 = _fold(p["mlp2"]["bn"])
    W2 = np.asarray(p["mlp2"]["W"]) * s2[:, None]           # [128, 128]
    b_f2 = W2 @ lin_b + t2                                  # [128]
    c["w2T"] = W2.T.astype(np.float16)
    # short
    ss, ts = _fold(p["short"]["bn"])
    Ws = np.asarray(p["short"]["W"]) * ss[:, None]          # [128, 32]
    c["wsT"] = Ws.T.astype(np.float16)                      # [32, 128]
    c["btot"] = (b_f2 + ts).astype(np.float32)              # [128]
    # mask / identity / ones
    m = np.zeros((128, 128), np.float16)
    for j in range(8):
        m[16 * j:16 * j + 16, 16 * j:16 * j + 16] = 1.0
    c["bdmask"] = m
    c["ident"] = np.eye(128, dtype=np.float16)
    c["ones"] = np.ones((1, 128), np.float16)
    return c


def _idx_tile(flat):
    """Flat int index list (len % 16 == 0) -> [128, len//16] int16 dma_gather layout."""
    arr = np.asarray(flat, np.int16).reshape(-1, 16).T      # [16, len//16]
    return np.tile(arr, (8, 1))                             # [128, len//16]


CONST_SPECS = [
    ("w1bd", [128, 128], F16), ("w1T", [32, 32], F16),
    ("fcbd", [128, 128], F16), ("wn1bd", [128, 128], F16), ("wn1T", [4, 8], F16),
    ("wn2bd", [128, 128], F16), ("w3rep", [8, 16], F16), ("bwn3row", [1, 16], F16),
    ("l1bd", [128, 128], F16), ("l2bd", [128, 128], F16), ("l3bd", [128, 128], F16),
    ("w4aug", [65, 128], F16), ("awt", [128, 2048], F16), ("w2T", [128, 128], F16),
    ("wsT", [32, 128], F16), ("bdmask", [128, 128], F16), ("ident", [128, 128], F16),
    ("ones", [1, 128], F16),
]
BIAS_SPECS = [
    ("b1", 128), ("b1s", 32), ("bwn1", 128), ("bwn2", 128),
    ("bL1", 128), ("bL2", 128), ("bL3", 128), ("btot", 128),
]


def build_nc():
    STAGE = int(os.environ.get("BK_STAGE", "9"))
    nc = bacc.Bacc("TRN2", target_bir_lowering=False, debug=False)
    tab_d = nc.dram_tensor("tab", [B, N, TABC], F16, kind="ExternalInput")
    nidx_d = nc.dram_tensor("nidx", [B, NT, 128, TP], I16, kind="ExternalInput")
    sidx_d = nc.dram_tensor("sidx", [128, NS // 16], I16, kind="ExternalInput")
    cd = {}
    for name, shape, dt in CONST_SPECS:
        cd[name] = nc.dram_tensor(name, shape, dt, kind="ExternalInput")
    for name, n in BIAS_SPECS:
        cd[name] = nc.dram_tensor(name, [n, 1], F32, kind="ExternalInput")
    out_d = nc.dram_tensor("out", [B, C2, NS], F32, kind="ExternalOutput")

    RELU = mybir.ActivationFunctionType.Relu
    COPY = mybir.ActivationFunctionType.Copy
    EXP = mybir.ActivationFunctionType.Exp
    LRELU = mybir.ActivationFunctionType.Lrelu
    SUB = mybir.AluOpType.subtract
    MUL = mybir.AluOpType.mult

    def bc3(ap, jn, kn):
        """[p, cols] AP -> [p, jn, kn] with k broadcast (step 0)."""
        return bass.AP(tensor=ap.tensor, offset=ap.offset,
                       ap=[ap.ap[0], [ap.ap[-1][0], jn], [0, kn]])

    with tile.TileContext(nc) as tc:
        with (
            tc.tile_pool(name="const", bufs=1) as cp,
            tc.tile_pool(name="batch", bufs=1) as bp,
            tc.tile_pool(name="gath", bufs=3) as gp,
            tc.tile_pool(name="work", bufs=2) as wp,
            tc.tile_pool(name="psT", bufs=2, space="PSUM") as psT,
            tc.tile_pool(name="psA", bufs=2, space="PSUM") as psA,
            tc.tile_pool(name="psB", bufs=2, space="PSUM") as psB,
            tc.tile_pool(name="psS", bufs=2, space="PSUM") as psS,
        ):
            ct = {}
            for name, shape, dt in CONST_SPECS:
                ct[name] = cp.tile(shape, dt, name="c_" + name)
                nc.sync.dma_start(out=ct[name][:], in_=cd[name][:])
            for name, n in BIAS_SPECS:
                ct[name] = cp.tile([n, 1], F32, name="c_" + name)
                nc.sync.dma_start(out=ct[name][:], in_=cd[name][:])

            # L4t stationary quarters with ones row (ping-pong pair)
            fr3q = [[cp.tile([65, QC], F16, name=f"fr3q{pp}{q}", tag=f"fr3q{pp}{q}") for q in range(4)]
                    for pp in range(2)]
            for pp in range(2):
                for q in range(4):
                    nc.vector.memset(fr3q[pp][q][64:65, :], 1.0)

            sidx_t = cp.tile([128, NS // 16], I16)
            nc.sync.dma_start(out=sidx_t[:], in_=sidx_d[:])

            NOBATCH = os.environ.get("BK_NOBATCH", "0") == "1"
            for b in range(B):
                if NOBATCH:
                    for st in range(NST):
                        otz = wp.tile([128, STT * TP], F32, tag="ot")
                        nc.vector.memset(otz[:], 0.0)
                        nc.sync.dma_start(out=out_d[b, :, 512 * st:512 * (st + 1)], in_=otz[:])
                    continue
                # ---- per-batch self prep ----
                gS = bp.tile([128, NT, TABC], F16, tag="gS")
                nc.gpsimd.dma_gather(
                    out_ap=gS[:], in_ap=tab_d[b], idxs_ap=sidx_t[:],
                    num_idxs=NS, num_idxs_reg=NS, elem_size=TABC, transpose=False,
                    single_packet=False)
                sf_feat = bp.tile([32, NS], F16, tag="sf_feat")
                sf_xyz = bp.tile([4, NS], F16, tag="sf_xyz")
                for q in range(NT):
                    tp_ = psT.tile([128, 128], F16, tag="t")
                    nc.tensor.transpose(tp_[:], gS[:, q, :], ct["ident"][:])
                    nc.scalar.activation(out=sf_feat[:, 128 * q:128 * (q + 1)],
                                         in_=tp_[0:32, :], func=COPY)
                    nc.vector.tensor_copy(out=sf_xyz[:, 128 * q:128 * (q + 1)],
                                          in_=tp_[32:36, :])
                # self pc: relu(W1 sf + b1)
                pcS = bp.tile([32, NS], F16, tag="pcS")
                for s_ in range(4):
                    pP = psS.tile([32, 512], F32, tag="s")
                    nc.tensor.matmul(pP[:], ct["w1T"][:], sf_feat[:, 512 * s_:512 * (s_ + 1)],
                                     start=True, stop=True)
                    nc.scalar.activation(out=pcS[:, 512 * s_:512 * (s_ + 1)], in_=pP[:],
                                         func=RELU, bias=ct["b1s"][:])
                # quarter-stacked self pc [128, (t:16, j:32)]
                pcS128 = bp.tile([128, QC], F16, tag="pcS128")
                for q in range(4):
                    src = pcS[:, 32 * q:]
                    ap = bass.AP(tensor=src.tensor, offset=src.offset,
                                 ap=[src.ap[0], [128, NT], [1, 32]])
                    nc.vector.tensor_copy(
                        out=pcS128[32 * q:32 * (q + 1), :].rearrange("p (t j) -> p t j", j=32),
                        in_=ap)
                # self wn1 pre-act: uS = wn1T.T? -> out = lhsT.T @ rhs with lhsT=wn1T [4,8]
                uS = bp.tile([8, NS], F16, tag="uS")
                for s_ in range(4):
                    uP = psS.tile([8, 512], F32, tag="s")
                    nc.tensor.matmul(uP[:], ct["wn1T"][:], sf_xyz[:, 512 * s_:512 * (s_ + 1)],
                                     start=True, stop=True)
                    nc.vector.tensor_copy(out=uS[:, 512 * s_:512 * (s_ + 1)], in_=uP[:])
                uS32 = bp.tile([128, QC], F16, tag="uS32")
                nc.vector.memset(uS32[:], 0.0)
                for q in range(4):
                    src = uS[:, 32 * q:]
                    ap = bass.AP(tensor=src.tensor, offset=src.offset,
                                 ap=[src.ap[0], [128, NT], [1, 32]])
                    nc.vector.tensor_copy(
                        out=uS32[32 * q:32 * q + 8, :].rearrange("p (t j) -> p t j", j=32),
                        in_=ap)

                for st in range(NST):
                    gbuf = wp.tile([128, STT * NKC], F16, tag="gbuf")
                    for tt in range(STT):
                        t = st * STT + tt
                        pp = t % 2
                        if STAGE < 1:
                            continue
                        # ---- gather ----
                        nidx_t = gp.tile([128, TP], I16, tag="nidx")
                        nc.sync.dma_start(out=nidx_t[:], in_=nidx_d[b, t])
                        gN = gp.tile([128, K, TABC], F16, tag="gN")
                        nc.gpsimd.dma_gather(
                            out_ap=gN[:], in_ap=tab_d[b], idxs_ap=nidx_t[:],
                            num_idxs=NKC, num_idxs_reg=NKC, elem_size=TABC,
                            transpose=False, single_packet=False)
                        # ---- transpose to channel-major (quarter-stacked) ----
                        fN128 = wp.tile([128, QC], F16, tag="fN128")
                        xyz128 = wp.tile([128, QC], F16, tag="xyz128")
                        for a in range(16):
                            tp_ = psT.tile([128, 128], F16, tag="t")
                            nc.tensor.transpose(tp_[:], gN[:, a, :], ct["ident"][:])
                            qtr, cb = a // 4, a % 4
                            nc.scalar.activation(
                                out=fN128[32 * qtr:32 * qtr + 32, 128 * cb:128 * cb + 128],
                                in_=tp_[0:32, :], func=COPY)
                            nc.vector.tensor_copy(
                                out=xyz128[32 * qtr:32 * qtr + 32, 128 * cb:128 * cb + 128],
                                in_=tp_[32:64, :])
                        if STAGE < 2:
                            continue
                        # ---- pc conv ----
                        pcP = psA.tile([128, QC], F32, tag="a")
                        nc.tensor.matmul(pcP[:], ct["w1bd"][:], fN128[:], start=True, stop=True)
                        pcN = wp.tile([128, QC], F16, tag="pcN")
                        nc.scalar.activation(out=pcN[:], in_=pcP[:], func=RELU, bias=ct["b1"][:])
                        # ---- rel_f = pcS_bcast - pcN ----
                        relf = wp.tile([128, QC], F16, tag="relf")
                        nc.vector.tensor_tensor(
                            out=relf[:].rearrange("p (j k) -> p j k", k=16),
                            in0=bc3(pcS128[:, 32 * t:32 * t + 32], 32, 16),
                            in1=pcN[:].rearrange("p (j k) -> p j k", k=16), op=SUB)
                        # ---- fc + softmax ----
                        zP = psA.tile([128, QC], F32, tag="a")
                        nc.tensor.matmul(zP[:], ct["fcbd"][:], relf[:], start=True, stop=True)
                        ex = wp.tile([128, QC], F32, tag="ex")
                        nc.scalar.activation(out=ex[:], in_=zP[:], func=EXP)
                        ssum = wp.tile([128, 32], F32, tag="ssum")
                        nc.vector.reduce_sum(out=ssum[:],
                                             in_=ex[:].rearrange("p (j k) -> p j k", k=16),
                                             axis=mybir.AxisListType.X)
                        rec = wp.tile([128, 32], F32, tag="rec")
                        nc.vector.reciprocal(out=rec[:], in_=ssum[:])
                        sm = wp.tile([128, QC], F16, tag="sm")
                        nc.vector.tensor_tensor(
                            out=sm[:].rearrange("p (j k) -> p j k", k=16),
                            in0=ex[:].rearrange("p (j k) -> p j k", k=16),
                            in1=bc3(rec[:], 32, 16), op=MUL)
                        frq = wp.tile([128, QC], F16, tag="frq")
                        nc.vector.tensor_tensor(out=frq[:], in0=sm[:], in1=relf[:], op=MUL)
                        if STAGE < 3:
                            continue
                        # ---- L1 ----
                        l1A = psA.tile([128, QC], F32, tag="a")
                        nc.tensor.matmul(l1A[:], ct["l1bd"][0:64, :], frq[0:64, :],
                                         start=True, stop=True)
                        l1B = psB.tile([128, QC], F32, tag="b")
                        nc.tensor.matmul(l1B[:], ct["l1bd"][64:128, :], frq[64:128, :],
                                         start=True, stop=True)
                        fr1A = wp.tile([128, QC], F16, tag="fr1A")
                        nc.scalar.activation(out=fr1A[:], in_=l1A[:], func=RELU, bias=ct["bL1"][:])
                        fr1B = wp.tile([128, QC], F16, tag="fr1B")
                        nc.scalar.activation(out=fr1B[:], in_=l1B[:], func=RELU, bias=ct["bL1"][:])
                        # ---- L2 ----
                        l2A = psA.tile([128, QC], F32, tag="a")
                        nc.tensor.matmul(l2A[:], ct["l2bd"][:], fr1A[:], start=True, stop=True)
                        l2B = psB.tile([128, QC], F32, tag="b")
                        nc.tensor.matmul(l2B[:], ct["l2bd"][:], fr1B[:], start=True, stop=True)
                        fr2A = wp.tile([128, QC], F16, tag="fr2A")
                        nc.scalar.activation(out=fr2A[:], in_=l2A[:], func=RELU, bias=ct["bL2"][:])
                        fr2B = wp.tile([128, QC], F16, tag="fr2B")
                        nc.scalar.activation(out=fr2B[:], in_=l2B[:], func=RELU, bias=ct["bL2"][:])
                        # ---- L3 -> fr3 quarters (with ones row) ----
                        l3A = psA.tile([128, QC], F32, tag="a")
                        nc.tensor.matmul(l3A[:], ct["l3bd"][:], fr2A[:], start=True, stop=True)
                        l3B = psB.tile([128, QC], F32, tag="b")
                        nc.tensor.matmul(l3B[:], ct["l3bd"][:], fr2B[:], start=True, stop=True)
                        nc.scalar.activation(out=fr3q[pp][0][0:64, :], in_=l3A[0:64, :],
                                             func=RELU, bias=ct["bL3"][0:64])
                        nc.scalar.activation(out=fr3q[pp][1][0:64, :], in_=l3A[64:128, :],
                                             func=RELU, bias=ct["bL3"][64:128])
                        nc.scalar.activation(out=fr3q[pp][2][0:64, :], in_=l3B[0:64, :],
                                             func=RELU, bias=ct["bL3"][0:64])
                        nc.scalar.activation(out=fr3q[pp][3][0:64, :], in_=l3B[64:128, :],
                                             func=RELU, bias=ct["bL3"][64:128])
                        if STAGE < 4:
                            continue
                        # ---- weightnet ----
                        uP = psS.tile([128, QC], F32, tag="s")
                        nc.tensor.matmul(uP[:], ct["wn1bd"][:], xyz128[:], start=True, stop=True)
                        v_ = wp.tile([128, QC], F32, tag="v_")
                        nc.vector.tensor_tensor(
                            out=v_[:].rearrange("p (j k) -> p j k", k=16),
                            in0=bc3(uS32[:, 32 * t:32 * t + 32], 32, 16),
                            in1=uP[:].rearrange("p (j k) -> p j k", k=16), op=SUB)
                        w1q = wp.tile([128, QC], F16, tag="w1q")
                        nc.scalar.activation(out=w1q[:], in_=v_[:], func=RELU, bias=ct["bwn1"][:])
                        w2P = psS.tile([128, QC], F32, tag="s")
                        nc.tensor.matmul(w2P[:], ct["wn2bd"][:], w1q[:], start=True, stop=True)
                        w2q = [wp.tile([8, QC], F16, name=f"w2q{q}", tag=f"w2q{q}") for q in range(4)]
                        for q in range(4):
                            nc.scalar.activation(out=w2q[q][:], in_=w2P[32 * q:32 * q + 8, :],
                                                 func=RELU, bias=ct["bwn2"][32 * q:32 * q + 8])
                        if STAGE < 5:
                            continue
                        # ---- per-group: wn3t, L4t, einsum ----
                        for a in range(16):
                            qtr, cb = a // 4, a % 4
                            w3P = psS.tile([128, 16], F32, tag="s")
                            nc.tensor.matmul(w3P[:], ct["ones"][:], ct["bwn3row"][:],
                                             start=True, stop=False)
                            nc.tensor.matmul(
                                w3P[:],
                                w2q[qtr][:, 128 * cb:128 * cb + 128],
                                ct["w3rep"][:],
                                start=False, stop=True)
                            wsm = wp.tile([128, 16], F16, tag="wsm")
                            nc.scalar.activation(out=wsm[:], in_=w3P[:], func=RELU)
                            bd_ = wp.tile([128, 128], F16, tag="bd_")
                            wr = wsm[:]
                            wrep = bass.AP(tensor=wr.tensor, offset=wr.offset,
                                           ap=[wr.ap[0], [0, 8], [wr.ap[-1][0], 16]])
                            nc.vector.tensor_tensor(
                                out=bd_[:].rearrange("p (j w) -> p j w", w=16),
                                in0=wrep,
                                in1=ct["bdmask"][:].rearrange("p (j w) -> p j w", w=16),
                                op=MUL)
                            # L4t
                            g4 = psT.tile([128, 128], F32, tag="t")
                            nc.tensor.matmul(
                                g4[:], fr3q[pp][qtr][:, 128 * cb:128 * cb + 128],
                                ct["w4aug"][:], start=True, stop=True)
                            fr4 = wp.tile([128, 128], F16, tag="fr4")
                            nc.scalar.activation(out=fr4[:], in_=g4[:], func=RELU)
                            # einsum
                            gP = psT.tile([128, 128], F32, tag="t")
                            nc.tensor.matmul(gP[:], fr4[:], bd_[:], start=True, stop=True)
                            nc.scalar.activation(
                                out=gbuf[:, NKC * tt + 128 * a: NKC * tt + 128 * a + 128],
                                in_=gP[:], func=COPY)
                    if STAGE < 6:
                        ot0 = wp.tile([128, STT * TP], F32, tag="ot")
                        nc.vector.memset(ot0[:], 0.0)
                        nc.sync.dma_start(out=out_d[b, :, 512 * st:512 * (st + 1)], in_=ot0[:])
                        continue
                    # ---- supertile: lin, mlp2 + short, output ----
                    nfP = psA.tile([128, STT * TP], F32, tag="a")
                    for w in range(16):
                        gb = gbuf[:, w:]
                        rhs = bass.AP(tensor=gb.tensor, offset=gb.offset,
                                      ap=[gb.ap[0], [128, 64], [16, 8]])
                        nc.tensor.matmul(nfP[:], ct["awt"][:, 128 * w:128 * w + 128], rhs,
                                         start=(w == 0), stop=(w == 15))
                    nf = wp.tile([128, STT * TP], F16, tag="nf")
                    nc.scalar.activation(out=nf[:], in_=nfP[:], func=COPY)
                    oP = psB.tile([128, STT * TP], F32, tag="b")
                    nc.tensor.matmul(oP[:], ct["w2T"][:], nf[:], start=True, stop=False)
                    nc.tensor.matmul(oP[:], ct["wsT"][:],
                                     sf_feat[0:32, 512 * st:512 * (st + 1)],
                                     start=False, stop=True)
                    y_ = wp.tile([128, STT * TP], F32, tag="y_")
                    nc.scalar.activation(out=y_[:], in_=oP[:],
                                         func=mybir.ActivationFunctionType.Identity,
                                         bias=ct["btot"][:])
                    y2 = wp.tile([128, STT * TP], F32, tag="y2")
                    nc.vector.tensor_scalar_mul(out=y2[:], in0=y_[:], scalar1=0.2)
                    ot = wp.tile([128, STT * TP], F32, tag="ot")
                    nc.vector.tensor_max(out=ot[:], in0=y_[:], in1=y2[:])
                    nc.sync.dma_start(out=out_d[b, :, 512 * st:512 * (st + 1)], in_=ot[:])
    nc.compile()
    return nc


def _host_prep(feature, xyz, neigh_idx, params):
    feature = np.asarray(feature)
    xyz = np.asarray(xyz)
    neigh_idx = np.asarray(neigh_idx)
    c = _prep_consts(params)
    tab = np.zeros((B, N, TABC), np.float16)
    for b in range(B):
        tab[b, :, 0:32] = feature[b, :, :, 0].T
        tab[b, :, 32:35] = xyz[b]
    base = {name: c[name] for name, _, _ in CONST_SPECS}
    for name, n in BIAS_SPECS:
        base[name] = c[name].reshape(n, 1)
    base["tab"] = tab
    base["sidx"] = _idx_tile(np.arange(NS))  # relative to shard? NO - table-global
    in_maps = []
    for core in range(NCORES):
        m = dict(base)
        p0 = core * NS
        m["sidx"] = _idx_tile(np.arange(p0, p0 + NS))
        nidx = np.empty((B, NT, 128, TP), np.int16)
        for b in range(B):
            for t in range(NT):
                pts = neigh_idx[b, p0 + 128 * t: p0 + 128 * (t + 1), :]  # [128, 16]
                nidx[b, t] = _idx_tile(pts.reshape(-1))
        m["nidx"] = nidx
        in_maps.append(m)
    return in_maps


_NC_CACHE = {}


def kernel(feature, xyz, neigh_idx, params):
    if "nc" not in _NC_CACHE:
        _NC_CACHE["nc"] = build_nc()
    nc = _NC_CACHE["nc"]
    in_maps = _host_prep(feature, xyz, neigh_idx, params)
    res = run_bass_kernel_spmd(nc, in_maps, core_ids=list(range(NCORES)))
    full = np.empty((B, C2, N, 1), np.float32)
    for core in range(NCORES):
        o = res.results[core]["out"]
        full[:, :, core * NS:(core + 1) * NS, 0] = o
    return full
